# revision 18
# baseline (speedup 1.0000x reference)
"""Trainium2 Bass kernel for nn_CMoSModel (moe_routing) - v2.

Data-parallel over batch: bs=256 -> 32 per core on 8 cores; params replicated.

Math (per row r=(b,c), L=512):
  mean/var over L; xn=(xt-mean)*rstd
  conv = depthwise(xn,k=16,s=8)+cb -> gates = top2-renorm softmax chain  [r,8]
  y_m = xn.seg @ map_w[m] + map_b[m];  out = (sum_m g_m y_m)*std + mean

Key identity used here: std*rstd = 1, so
  out[r,(o,s)] = sum_m g_m * yraw_m[r,(o,s)] + C[r,o]
  yraw_m = raw xt through expert weights (no normalize!)
  C[r,o] = -mean*sum_m g_m W1[m,o] + std*sum_m g_m b_m[o] + mean,
  W1[m,o] = sum_n map_w[m,o,n]

Structure (per core, 16 tiles of 128 rows = 2 batches x 64 ch):
  Pass A: load + PE-transpose input -> xt (bf16); bn_stats/aggr stats;
          depthwise conv as ONE windowed tensor_tensor (GPSIMD) + grouped
          tensor_reduce (DVE); Act writes permuted bf16 xnp; XBAR
          dma_start_transpose makes xnpT (matmul lhsT) - no PSUM round trip.
  I1/I2 (batched over all 16 tiles in single wide ops): std/rstd; gate
          logits += gb; double softmax + top-2 via grouped reduces and
          stride-0 broadcast APs; gsm17; per-tile correction C via tiny
          PE matmuls (17x45 rhs).
  Pass C: per tile 16 big matmuls (block-diag W4 [128,4*360] bf16, 360 cols
          each, full-K contraction) -> dense y in PSUM; combine = 8
          scalar_tensor_tensor madds per q split DVE/GPSIMD; output PE
          transposes reuse the same 8-bank PSUM pool; stores via gpsimd DMA.
"""

import sys

import numpy as np

for p in ("/opt/trn_rl_repo", "/opt/pypackages"):
    if p not in sys.path:
        sys.path.insert(0, p)

BS = 256
SEQ = 512
PRED = 720
C = 64
SEG = 16
NM = 8
KSZ = 16
STRIDE = 8
CONV_DIM = 63
N_IN = 32
N_OUT = 45
NCORES = 8
BPC = BS // NCORES   # 32 batches per core
NT = BPC // 2        # 16 tiles, 2 batches each

_CACHE = {}


def _win_ap(bass, tile_ap, col_off, part_stride):
    """Overlapping conv window AP: [p, d=63 (stride 8), k=16 (stride 1)]."""
    return bass.AP(
        tile_ap.tensor,
        tile_ap.offset + col_off,
        [[part_stride, 128], [STRIDE, CONV_DIM], [1, KSZ]],
    )


def _build_program():
    import concourse.bass as bass
    import concourse.tile as tile
    from concourse import bacc
    from concourse import mybir
    from concourse.masks import make_identity

    f32 = mybir.dt.float32
    bf16 = mybir.dt.bfloat16
    AL = mybir.AluOpType
    AF = mybir.ActivationFunctionType
    AX = mybir.AxisListType

    nc = bacc.Bacc(None, target_bir_lowering=False)
    x_d = nc.declare_dram_parameter("x", [BPC, SEQ, C], f32, isOutput=False)
    cw_d = nc.declare_dram_parameter("conv_w", [C, 1, KSZ], f32, isOutput=False)
    cb_d = nc.declare_dram_parameter("conv_b", [C], f32, isOutput=False)
    gw_d = nc.declare_dram_parameter("gate_w", [NM, CONV_DIM], f32, isOutput=False)
    gb_d = nc.declare_dram_parameter("gate_b", [NM], f32, isOutput=False)
    mw_d = nc.declare_dram_parameter("map_w", [NM, N_OUT, N_IN], f32, isOutput=False)
    mb_d = nc.declare_dram_parameter("map_b", [NM, N_OUT], f32, isOutput=False)
    out_d = nc.declare_dram_parameter("out", [BPC, PRED, C], f32, isOutput=True)

    with tile.TileContext(nc) as tc:
        with (
            tc.tile_pool(name="consts", bufs=1) as consts,
            tc.tile_pool(name="big", bufs=1) as big,
            tc.tile_pool(name="xin", bufs=3) as xin,
            tc.tile_pool(name="cvp", bufs=3) as cvp,
            tc.tile_pool(name="small", bufs=3) as small,
            tc.tile_pool(name="accp", bufs=2) as accp,
            tc.tile_pool(name="ocsp", bufs=3) as ocsp,
        ):
            # ---------------- constants ----------------
            zero_t = consts.tile([128, 1], f32)
            nc.gpsimd.memset(zero_t[:], 0.0)
            nc.const_aps.aps[(f32, 0.0)] = zero_t[:]

            ident_f = consts.tile([128, 128], f32)
            make_identity(nc, ident_f[:])
            ident_m = consts.tile([128, 128], bf16)
            make_identity(nc, ident_m[:])

            # conv weights per-channel, dup 2x over h
            cw_t = consts.tile([128, KSZ], f32)
            nc.sync.dma_start(cw_t[0:64, :], cw_d[:, 0, :])
            nc.sync.dma_start(cw_t[64:128, :], cw_d[:, 0, :])
            cb_t = consts.tile([128, 1], f32)
            nc.sync.dma_start(cb_t[0:64, :], cb_d[:, None])
            nc.sync.dma_start(cb_t[64:128, :], cb_d[:, None])
            negSw = consts.tile([128, 1], f32)
            nc.vector.tensor_reduce(negSw[:], cw_t[:], axis=AX.X, op=AL.add)
            nc.vector.tensor_scalar(negSw[:], negSw[:], -1.0, None, AL.mult)

            # gate weights: gstack [64, 8] = [gw.T (63 rows); Gsum (1 row)]
            gw_f = consts.tile([CONV_DIM, NM], f32)
            nc.sync.dma_start(gw_f[:, :], gw_d[:].rearrange("m d -> d m"))
            gstack = consts.tile([64, NM], f32)
            nc.vector.tensor_copy(gstack[0:CONV_DIM, :], gw_f[:])
            gw_row = consts.tile([1, NM * CONV_DIM], f32)
            nc.sync.dma_start(
                gw_row[:, :], gw_d[:].rearrange("m d -> (m d)")[None, :]
            )
            gsum_r = consts.tile([1, NM], f32)
            nc.vector.tensor_reduce(
                gsum_r[:],
                gw_row[:].rearrange("p (m d) -> p m d", m=NM),
                axis=AX.X, op=AL.add,
            )
            nc.sync.dma_start(gstack[CONV_DIM:64, :], gsum_r[:])

            gb8 = consts.tile([128, NM], f32)
            nc.sync.dma_start(gb8[:, :], gb_d[None, :].broadcast_to([128, NM]))

            # W4 block-diag [128=(s,n), 4*360=(s',m,o)] bf16
            W4f = consts.tile([128, 4 * NM * N_OUT], f32)
            nc.vector.memset(W4f[:], 0.0)
            for s in range(4):
                nc.sync.dma_start(
                    W4f[32 * s : 32 * (s + 1),
                        360 * s : 360 * (s + 1)],
                    mw_d[:].rearrange("m o n -> n (m o)"),
                )
            W4 = consts.tile([128, 4 * NM * N_OUT], bf16)
            nc.vector.tensor_copy(W4[:], W4f[:])

            # Crhs [17, 45] = [W1 (8); map_b (8); ones (1)]
            mwn = consts.tile([NM, N_OUT * N_IN], f32)
            nc.sync.dma_start(mwn[:, :], mw_d[:].rearrange("m o n -> m (o n)"))
            Crhs_f = consts.tile([17, N_OUT], f32)
            nc.vector.memset(Crhs_f[:], 1.0)
            nc.vector.tensor_reduce(
                Crhs_f[0:NM, :],
                mwn[:].rearrange("p (o n) -> p o n", o=N_OUT),
                axis=AX.X, op=AL.add,
            )
            nc.sync.dma_start(Crhs_f[NM : 2 * NM, :], mb_d[:, :])
            Crhsb = consts.tile([17, N_OUT], bf16)
            nc.vector.tensor_copy(Crhsb[:], Crhs_f[:])

            # ---------------- big SBUF tensors ----------------
            xtALL = big.tile([128, NT * SEQ], f32)       # [r, (t, l)]
            xnpALL = big.tile([128, NT * SEQ], bf16)      # [r, (t, q, s, n)]
            xnpT = big.tile([128, NT * 4 * 128], bf16)    # [(s,n), (t, q, r)]
            statsALL = big.tile([128, NT * 2], f32)       # (t, [mean, var])
            convrawALL = big.tile([128, NT * CONV_DIM], f32)
            convsALL = big.tile([128, NT * 64], f32)     # [r, (t, 64)]
            cvTALL = big.tile([64, NT * 128], f32)
            gsmTALL = big.tile([17, NT * 128], bf16)
            CsbALL = big.tile([128, NT * N_OUT], f32)
            gALL = big.tile([128, NT * NM], f32)
            lgALL = big.tile([128, NT * NM], f32)
            E1 = big.tile([128, NT * NM], f32)
            vt = big.tile([128, NT * NM], f32)
            E2 = big.tile([128, NT * NM], f32)
            EmALL = big.tile([128, NT * NM], f32)
            gsm17 = big.tile([128, NT * 17], f32)
            gsm17b = big.tile([128, NT * 17], bf16)
            st16a = big.tile([128, NT], f32)   # vstab
            stdALL = big.tile([128, NT], f32)
            rstdALL = big.tile([128, NT], f32)
            negmean = big.tile([128, NT], f32)
            s1g = big.tile([128, NT], f32)
            r1g = big.tile([128, NT], f32)
            mx1 = big.tile([128, NT], f32)
            mx2 = big.tile([128, NT], f32)
            E2m = big.tile([128, NT * NM], f32)
            E2b = big.tile([128, NT * NM], f32)
            msk1 = big.tile([128, NT * NM], f32)
            mskf = big.tile([128, NT * NM], f32)
            tmp16 = big.tile([128, NT], f32)

            meanv = statsALL[:].rearrange("p (t u) -> p t u", u=2)[:, :, 0]
            varv = statsALL[:].rearrange("p (t u) -> p t u", u=2)[:, :, 1]

            HT = NT // 2  # tiles per half

            def emit_A(t, pA):
                xts = xtALL[:, t * SEQ : (t + 1) * SEQ]
                xraw = xin.tile([128, 2 * 4 * C], f32, tag="xraw")
                xrv = xraw[:].rearrange("p (h j c) -> p h j c", h=2, j=4)
                nc.sync.dma_start(
                    xrv,
                    x_d[2 * t : 2 * t + 2].rearrange(
                        "h (j p) c -> p h j c", p=128
                    ),
                )
                for h in range(2):
                    psx = pA.tile([64, SEQ], f32, tag="psx")
                    for j in range(4):
                        nc.tensor.transpose(
                            psx[:, j * 128 : (j + 1) * 128], xrv[:, h, j],
                            ident_f[:],
                        )
                    if h == 0:
                        nc.vector.tensor_copy(xts[0:64, :], psx[:])
                    else:
                        nc.scalar.copy(xts[64:128, :], psx[:])

                bs6 = small.tile([128, 6], f32, tag="bs6")
                nc.vector.bn_stats(bs6[:], xts)
                nc.vector.bn_aggr(statsALL[:, 2 * t : 2 * t + 2], bs6[:])

                cvt = cvp.tile([128, CONV_DIM * KSZ], f32, tag="cvt")
                nc.gpsimd.tensor_tensor(
                    cvt[:].rearrange("p (d k) -> p d k", k=KSZ),
                    _win_ap(bass, xtALL[:], t * SEQ, NT * SEQ),
                    cw_t[:].unsqueeze(1).broadcast_to([128, CONV_DIM, KSZ]),
                    AL.mult,
                )
                nc.vector.tensor_reduce(
                    convrawALL[:, t * CONV_DIM : (t + 1) * CONV_DIM],
                    cvt[:].rearrange("p (d k) -> p d k", k=KSZ),
                    axis=AX.X, op=AL.add,
                )

                nc.scalar.copy(
                    xnpALL[:, t * SEQ : (t + 1) * SEQ].rearrange(
                        "p (q s n) -> p q s n", q=4, s=4
                    ),
                    xts.rearrange("p (n q s) -> p n q s", n=N_IN, q=4)
                    .rearrange("p n q s -> p q s n"),
                )
                nc.scalar.dma_start_transpose(
                    xnpT[:, t * SEQ : (t + 1) * SEQ].rearrange(
                        "p (q r) -> p q r", q=4
                    ),
                    xnpALL[:, t * SEQ : (t + 1) * SEQ],
                )

            def emit_gates(half):
                ts, te = half * HT, (half + 1) * HT
                tsl = slice(ts, te)
                sl8 = slice(ts * NM, te * NM)
                meanh = statsALL[:].rearrange("p (t u) -> p t u", u=2)[:, tsl, 0]
                varh = statsALL[:].rearrange("p (t u) -> p t u", u=2)[:, tsl, 1]
                sth = st16a[:, tsl]
                nc.vector.tensor_scalar(sth, varh, 1e-10, None, AL.add)
                nc.scalar.activation(stdALL[:, tsl], sth, AF.Sqrt)
                nc.vector.reciprocal(rstdALL[:, tsl], stdALL[:, tsl])
                nc.vector.tensor_scalar(
                    negmean[:, tsl], meanh, -1.0, None, AL.mult
                )

                convs_v = convsALL[:].rearrange("p (t u) -> p t u", u=64)[:, tsl]
                nc.vector.tensor_tensor(
                    convs_v[:, :, 0:CONV_DIM],
                    convrawALL[:].rearrange("p (t d) -> p t d", d=CONV_DIM)[
                        :, tsl
                    ],
                    rstdALL[:, tsl].unsqueeze(-1).broadcast_to(
                        [128, HT, CONV_DIM]
                    ),
                    AL.mult,
                )
                nc.vector.tensor_tensor(
                    tmp16[:, tsl], meanh, rstdALL[:, tsl], AL.mult
                )
                nc.vector.scalar_tensor_tensor(
                    convs_v[:, :, CONV_DIM : CONV_DIM + 1].squeeze(-1),
                    tmp16[:, tsl], negSw[:],
                    cb_t[:].broadcast_to([128, HT]),
                    AL.mult, AL.add,
                )
                with tc.tile_pool(name="pgate", bufs=1, space="PSUM") as pgate:
                    gpsum = pgate.tile([128, HT * NM], f32, name=f"gp{half}")
                    with tc.tile_pool(name="pB", bufs=1, space="PSUM") as pB:
                        psB = pB.tile([64, HT * 128], f32, name=f"psB{half}")
                        for i, t in enumerate(range(ts, te)):
                            nc.tensor.transpose(
                                psB[:, i * 128 : (i + 1) * 128],
                                convsALL[:, t * 64 : (t + 1) * 64],
                                ident_f[:],
                            )
                        nc.vector.tensor_copy(
                            cvTALL[:, ts * 128 : te * 128], psB[:]
                        )
                        for i, t in enumerate(range(ts, te)):
                            nc.tensor.matmul(
                                gpsum[:, i * NM : (i + 1) * NM],
                                cvTALL[:, t * 128 : (t + 1) * 128],
                                gstack[:],
                                start=True, stop=True,
                            )
                    lg_v = lgALL[:, sl8].rearrange("p (t m) -> p t m", m=NM)
                    nc.vector.tensor_tensor(
                        lg_v, gpsum[:].rearrange("p (t m) -> p t m", m=NM),
                        gb8[:].unsqueeze(1).broadcast_to([128, HT, NM]),
                        AL.add,
                    )
                nc.scalar.activation(E1[:, sl8], lgALL[:, sl8], AF.Exp)
                E1_v = E1[:, sl8].rearrange("p (t m) -> p t m", m=NM)
                nc.vector.tensor_reduce(s1g[:, tsl], E1_v, axis=AX.X, op=AL.add)
                nc.vector.reciprocal(r1g[:, tsl], s1g[:, tsl])
                vt_v = vt[:, sl8].rearrange("p (t m) -> p t m", m=NM)
                nc.vector.tensor_tensor(
                    vt_v, E1_v,
                    r1g[:, tsl].unsqueeze(-1).broadcast_to([128, HT, NM]),
                    AL.mult,
                )
                nc.scalar.activation(E2[:, sl8], vt[:, sl8], AF.Exp)
                E2_v = E2[:, sl8].rearrange("p (t m) -> p t m", m=NM)
                nc.vector.tensor_reduce(mx1[:, tsl], E2_v, axis=AX.X, op=AL.max)
                msk1_v = msk1[:, sl8].rearrange("p (t m) -> p t m", m=NM)
                nc.vector.tensor_tensor(
                    msk1_v, E2_v,
                    mx1[:, tsl].unsqueeze(-1).broadcast_to([128, HT, NM]),
                    AL.is_ge,
                )
                E2m_v = E2m[:, sl8].rearrange("p (t m) -> p t m", m=NM)
                nc.vector.tensor_tensor(E2m_v, E2_v, msk1_v, AL.mult)
                E2b_v = E2b[:, sl8].rearrange("p (t m) -> p t m", m=NM)
                nc.vector.tensor_tensor(E2b_v, E2_v, E2m_v, AL.subtract)
                nc.vector.tensor_reduce(mx2[:, tsl], E2b_v, axis=AX.X, op=AL.max)
                mskf_v = mskf[:, sl8].rearrange("p (t m) -> p t m", m=NM)
                nc.vector.tensor_tensor(
                    mskf_v, E2_v,
                    mx2[:, tsl].unsqueeze(-1).broadcast_to([128, HT, NM]),
                    AL.is_ge,
                )
                Em_v = EmALL[:, sl8].rearrange("p (t m) -> p t m", m=NM)
                nc.vector.tensor_tensor(Em_v, E2_v, mskf_v, AL.mult)
                nc.vector.tensor_reduce(s1g[:, tsl], Em_v, axis=AX.X, op=AL.add)
                nc.vector.reciprocal(r1g[:, tsl], s1g[:, tsl])
                g_v = gALL[:, sl8].rearrange("p (t m) -> p t m", m=NM)
                nc.vector.tensor_tensor(
                    g_v, Em_v,
                    r1g[:, tsl].unsqueeze(-1).broadcast_to([128, HT, NM]),
                    AL.mult,
                )
                g17 = gsm17[:].rearrange("p (t u) -> p t u", u=17)[:, tsl]
                nc.vector.tensor_tensor(
                    g17[:, :, 0:NM], g_v,
                    negmean[:, tsl].unsqueeze(-1).broadcast_to([128, HT, NM]),
                    AL.mult,
                )
                nc.vector.tensor_tensor(
                    g17[:, :, NM : 2 * NM], g_v,
                    stdALL[:, tsl].unsqueeze(-1).broadcast_to([128, HT, NM]),
                    AL.mult,
                )
                nc.vector.tensor_copy(
                    g17[:, :, 16:17],
                    statsALL[:].rearrange("p (t u) -> p t u", u=2)[:, tsl, 0:1],
                )
                nc.vector.tensor_copy(
                    gsm17b[:, ts * 17 : te * 17], gsm17[:, ts * 17 : te * 17]
                )
                with tc.tile_pool(name="pI2", bufs=1, space="PSUM") as pI2:
                    psI = pI2.tile([17, HT * 128], bf16, name=f"psI{half}")
                    for i, t in enumerate(range(ts, te)):
                        nc.tensor.transpose(
                            psI[:, i * 128 : (i + 1) * 128],
                            gsm17b[:, t * 17 : (t + 1) * 17],
                            ident_m[:],
                        )
                    nc.vector.tensor_copy(
                        gsmTALL[:, ts * 128 : te * 128], psI[:]
                    )
                    Cps = pI2.tile([128, HT * N_OUT], f32, name=f"Cps{half}")
                    for i, t in enumerate(range(ts, te)):
                        nc.tensor.matmul(
                            Cps[:, i * N_OUT : (i + 1) * N_OUT],
                            gsmTALL[:, t * 128 : (t + 1) * 128],
                            Crhsb[:],
                            start=True, stop=True,
                        )
                    nc.scalar.copy(
                        CsbALL[:, ts * N_OUT : te * N_OUT], Cps[:]
                    )

            def emit_experts(t, qp):
                # 8 psum units of 2 segments each: unit u2 covers u = 2u2, 2u2+1
                acc = accp.tile([128, PRED], f32, tag="acc", bufs=3)
                ysb = accp.tile([128, 4 * 4 * 360], bf16, tag="ysb", bufs=4)
                for u2 in range(8):
                    q = u2 // 2
                    yq = qp.tile([128, 2, 512], f32, tag="q")
                    lhsT = xnpT[:, (t * 4 + q) * 128 : (t * 4 + q + 1) * 128]
                    for sp in range(2):
                        s_loc = 2 * (u2 % 2) + sp
                        nc.tensor.matmul(
                            yq[:, sp, 0 : NM * N_OUT],
                            lhsT,
                            W4[:, s_loc * 360 : (s_loc + 1) * 360],
                            start=True, stop=True,
                        )
                    nc.scalar.copy(
                        ysb[:, u2 * 720 : (u2 + 1) * 720].rearrange(
                            "p (s mo) -> p s mo", s=2
                        ),
                        yq[:, :, 0:360],
                    )
                return acc, ysb

            def emit_madd(t, acc, ysb, m):
                # acc[p, 45u+o] = sum_m g_m ysb[p, 360u+45m+o] + C[p,o]
                dst = bass.AP(
                    acc.tensor, acc.offset,
                    [[PRED, 128], [N_OUT, SEG], [1, N_OUT]],
                )
                src = bass.AP(
                    ysb.tensor, ysb.offset + m * N_OUT,
                    [[4 * 1440, 128], [360, SEG], [1, N_OUT]],
                )
                gm = gALL[:, t * NM + m : t * NM + m + 1]
                if m == 0:
                    Cb = (
                        CsbALL[:, t * N_OUT : (t + 1) * N_OUT]
                        .unsqueeze(1)
                        .broadcast_to([128, SEG, N_OUT])
                    )
                    nc.vector.scalar_tensor_tensor(
                        dst, src, gm, Cb, AL.mult, AL.add
                    )
                else:
                    nc.vector.scalar_tensor_tensor(
                        dst, src, gm, dst, AL.mult, AL.add
                    )

            def emit_out(t, acc, qp):
                # 8 transposes of 90-col blocks: block b = segments (2b, 2b+1)
                pox = qp.tile([128, 2, 512], f32, tag="q")
                poxv = bass.AP(
                    pox.tensor, pox.offset, [[1024, 90], [128, 8], [1, 128]]
                )
                accv = acc[:].rearrange("p (b w) -> p b w", b=8)
                for b in range(8):
                    nc.tensor.transpose(
                        poxv[:, b : b + 1].squeeze(1), accv[:, b], ident_f[:]
                    )
                ocs = ocsp.tile([90, 8 * 128], f32, tag="ocs")
                nc.scalar.copy(ocs[:, 0:512], poxv[:, 0:4])
                nc.scalar.copy(ocs[:, 512:1024], poxv[:, 4:8])
                ocsv = ocs[:].rearrange("p (b h c) -> p b h c", b=8, h=2)
                for h in range(2):
                    dstv = out_d[2 * t + h].rearrange(
                        "(o b u) c -> o b u c", o=N_OUT, b=8
                    )
                    for u2 in range(2):
                        nc.sync.dma_start(
                            dstv[:, :, u2, :],
                            ocsv[45 * u2 : 45 * (u2 + 1), :, h, :],
                        )

            def emit_C_pair(t0, t1, qp):
                acc0, ysb0 = emit_experts(t0, qp)
                acc1, ysb1 = emit_experts(t1, qp)
                for m in range(NM):
                    emit_madd(t0, acc0, ysb0, m)
                    emit_madd(t1, acc1, ysb1, m)
                emit_out(t0, acc0, qp)
                emit_out(t1, acc1, qp)

            # ================= schedule =================
            with tc.tile_pool(name="pA", bufs=2, space="PSUM") as pA:
                for t in range(HT):
                    emit_A(t, pA)
                emit_gates(0)
                with tc.tile_pool(name="qp0", bufs=3, space="PSUM") as qp0:
                    for pp in range(HT // 2):
                        emit_C_pair(2 * pp, 2 * pp + 1, qp0)
                        emit_A(HT + 2 * pp, pA)
                        emit_A(HT + 2 * pp + 1, pA)
                emit_gates(1)
                with tc.tile_pool(name="qp1", bufs=3, space="PSUM") as qp1:
                    for pp in range(HT // 2):
                        emit_C_pair(HT + 2 * pp, HT + 2 * pp + 1, qp1)

    nc.compile()
    return nc


def _get_program():
    if "v2" not in _CACHE:
        _CACHE["v2"] = _build_program()
    return _CACHE["v2"]


def kernel(x, conv_w, conv_b, gate_w, gate_b, map_w, map_b, _mm_dt="bfloat16",
           _trace=False):
    from concourse.bass_utils import run_bass_kernel_spmd

    nc = _get_program()
    x = np.ascontiguousarray(np.asarray(x, dtype=np.float32))
    params = dict(
        conv_w=np.ascontiguousarray(np.asarray(conv_w, np.float32)),
        conv_b=np.ascontiguousarray(np.asarray(conv_b, np.float32)),
        gate_w=np.ascontiguousarray(np.asarray(gate_w, np.float32)),
        gate_b=np.ascontiguousarray(np.asarray(gate_b, np.float32)),
        map_w=np.ascontiguousarray(np.asarray(map_w, np.float32)),
        map_b=np.ascontiguousarray(np.asarray(map_b, np.float32)),
    )
    in_maps = [
        dict(x=x[i * BPC : (i + 1) * BPC], **params) for i in range(NCORES)
    ]
    res = run_bass_kernel_spmd(
        nc, in_maps, core_ids=list(range(NCORES)), trace=_trace
    )
    out = np.concatenate([res.results[i]["out"] for i in range(NCORES)], axis=0)
    if _trace:
        return out, res
    return out


# revision 22
# speedup vs baseline: 1.1698x; 1.1698x over previous
"""Trainium2 Bass kernel for nn_CMoSModel (moe_routing) - v2.

Data-parallel over batch: bs=256 -> 32 per core on 8 cores; params replicated.

Math (per row r=(b,c), L=512):
  mean/var over L; xn=(xt-mean)*rstd
  conv = depthwise(xn,k=16,s=8)+cb -> gates = top2-renorm softmax chain  [r,8]
  y_m = xn.seg @ map_w[m] + map_b[m];  out = (sum_m g_m y_m)*std + mean

Key identity used here: std*rstd = 1, so
  out[r,(o,s)] = sum_m g_m * yraw_m[r,(o,s)] + C[r,o]
  yraw_m = raw xt through expert weights (no normalize!)
  C[r,o] = -mean*sum_m g_m W1[m,o] + std*sum_m g_m b_m[o] + mean,
  W1[m,o] = sum_n map_w[m,o,n]

Structure (per core, 16 tiles of 128 rows = 2 batches x 64 ch):
  Pass A: load + PE-transpose input -> xt (bf16); bn_stats/aggr stats;
          depthwise conv as ONE windowed tensor_tensor (GPSIMD) + grouped
          tensor_reduce (DVE); Act writes permuted bf16 xnp; XBAR
          dma_start_transpose makes xnpT (matmul lhsT) - no PSUM round trip.
  I1/I2 (batched over all 16 tiles in single wide ops): std/rstd; gate
          logits += gb; double softmax + top-2 via grouped reduces and
          stride-0 broadcast APs; gsm17; per-tile correction C via tiny
          PE matmuls (17x45 rhs).
  Pass C: per tile 16 big matmuls (block-diag W4 [128,4*360] bf16, 360 cols
          each, full-K contraction) -> dense y in PSUM; combine = 8
          scalar_tensor_tensor madds per q split DVE/GPSIMD; output PE
          transposes reuse the same 8-bank PSUM pool; stores via gpsimd DMA.
"""

import sys

import numpy as np

for p in ("/opt/trn_rl_repo", "/opt/pypackages"):
    if p not in sys.path:
        sys.path.insert(0, p)

BS = 256
SEQ = 512
PRED = 720
C = 64
SEG = 16
NM = 8
KSZ = 16
STRIDE = 8
CONV_DIM = 63
N_IN = 32
N_OUT = 45
NCORES = 8
BPC = BS // NCORES   # 32 batches per core
NT = BPC // 2        # 16 tiles, 2 batches each

_CACHE = {}


def _win_ap(bass, tile_ap, col_off, part_stride):
    """Overlapping conv window AP: [p, d=63 (stride 8), k=16 (stride 1)]."""
    return bass.AP(
        tile_ap.tensor,
        tile_ap.offset + col_off,
        [[part_stride, 128], [STRIDE, CONV_DIM], [1, KSZ]],
    )


def _build_program():
    import concourse.bass as bass
    import concourse.tile as tile
    from concourse import bacc
    from concourse import mybir
    from concourse.masks import make_identity

    f32 = mybir.dt.float32
    bf16 = mybir.dt.bfloat16
    AL = mybir.AluOpType
    AF = mybir.ActivationFunctionType
    AX = mybir.AxisListType

    nc = bacc.Bacc(None, target_bir_lowering=False)
    x_d = nc.declare_dram_parameter("x", [BPC, SEQ, C], f32, isOutput=False)
    cw_d = nc.declare_dram_parameter("conv_w", [C, 1, KSZ], f32, isOutput=False)
    cb_d = nc.declare_dram_parameter("conv_b", [C], f32, isOutput=False)
    gw_d = nc.declare_dram_parameter("gate_w", [NM, CONV_DIM], f32, isOutput=False)
    gb_d = nc.declare_dram_parameter("gate_b", [NM], f32, isOutput=False)
    mw_d = nc.declare_dram_parameter("map_w", [NM, N_OUT, N_IN], f32, isOutput=False)
    mb_d = nc.declare_dram_parameter("map_b", [NM, N_OUT], f32, isOutput=False)
    out_d = nc.declare_dram_parameter("out", [BPC, PRED, C], f32, isOutput=True)

    with tile.TileContext(nc) as tc:
        with (
            tc.tile_pool(name="consts", bufs=1) as consts,
            tc.tile_pool(name="big", bufs=1) as big,
            tc.tile_pool(name="xin", bufs=3) as xin,
            tc.tile_pool(name="cvp", bufs=2) as cvp,
            tc.tile_pool(name="small", bufs=3) as small,
            tc.tile_pool(name="accp", bufs=2) as accp,
            tc.tile_pool(name="ocsp", bufs=3) as ocsp,
        ):
            # ---------------- constants ----------------
            zero_t = consts.tile([128, 1], f32)
            nc.gpsimd.memset(zero_t[:], 0.0)
            nc.const_aps.aps[(f32, 0.0)] = zero_t[:]

            ident_f = consts.tile([128, 128], f32)
            make_identity(nc, ident_f[:])
            ident_m = consts.tile([128, 128], bf16)
            make_identity(nc, ident_m[:])

            # conv weights per-channel, dup 2x over h
            cw_t = consts.tile([128, KSZ], f32)
            nc.sync.dma_start(cw_t[0:64, :], cw_d[:, 0, :])
            nc.sync.dma_start(cw_t[64:128, :], cw_d[:, 0, :])
            cb_t = consts.tile([128, 1], f32)
            nc.sync.dma_start(cb_t[0:64, :], cb_d[:, None])
            nc.sync.dma_start(cb_t[64:128, :], cb_d[:, None])
            negSw = consts.tile([128, 1], f32)
            nc.vector.tensor_reduce(negSw[:], cw_t[:], axis=AX.X, op=AL.add)
            nc.vector.tensor_scalar(negSw[:], negSw[:], -1.0, None, AL.mult)

            # gate weights: gstack [64, 8] = [gw.T (63 rows); Gsum (1 row)]
            gw_f = consts.tile([CONV_DIM, NM], f32)
            nc.sync.dma_start(gw_f[:, :], gw_d[:].rearrange("m d -> d m"))
            gstack = consts.tile([64, NM], f32)
            nc.vector.tensor_copy(gstack[0:CONV_DIM, :], gw_f[:])
            gw_row = consts.tile([1, NM * CONV_DIM], f32)
            nc.sync.dma_start(
                gw_row[:, :], gw_d[:].rearrange("m d -> (m d)")[None, :]
            )
            gsum_r = consts.tile([1, NM], f32)
            nc.vector.tensor_reduce(
                gsum_r[:],
                gw_row[:].rearrange("p (m d) -> p m d", m=NM),
                axis=AX.X, op=AL.add,
            )
            nc.sync.dma_start(gstack[CONV_DIM:64, :], gsum_r[:])

            gb8 = consts.tile([128, NM], f32)
            nc.sync.dma_start(gb8[:, :], gb_d[None, :].broadcast_to([128, NM]))

            # W4 block-diag [128=(s,n), 4*360=(s',m,o)] bf16
            W4f = consts.tile([128, 4 * NM * N_OUT], f32)
            nc.vector.memset(W4f[:], 0.0)
            for s in range(4):
                nc.sync.dma_start(
                    W4f[32 * s : 32 * (s + 1),
                        360 * s : 360 * (s + 1)],
                    mw_d[:].rearrange("m o n -> n (m o)"),
                )
            W4 = consts.tile([128, 4 * NM * N_OUT], bf16)
            nc.vector.tensor_copy(W4[:], W4f[:])

            # Crhs [17, 45] = [W1 (8); map_b (8); ones (1)]
            mwn = consts.tile([NM, N_OUT * N_IN], f32)
            nc.sync.dma_start(mwn[:, :], mw_d[:].rearrange("m o n -> m (o n)"))
            Crhs_f = consts.tile([17, N_OUT], f32)
            nc.vector.memset(Crhs_f[:], 1.0)
            nc.vector.tensor_reduce(
                Crhs_f[0:NM, :],
                mwn[:].rearrange("p (o n) -> p o n", o=N_OUT),
                axis=AX.X, op=AL.add,
            )
            nc.sync.dma_start(Crhs_f[NM : 2 * NM, :], mb_d[:, :])
            Crhsb = consts.tile([17, N_OUT], bf16)
            nc.vector.tensor_copy(Crhsb[:], Crhs_f[:])

            # ---------------- big SBUF tensors ----------------
            xtALL = big.tile([128, NT * SEQ], f32)       # [r, (t, l)]
            xnpALL = big.tile([128, NT * SEQ], bf16)      # [r, (t, q, s, n)]
            xnpT = big.tile([128, NT * 4 * 128], bf16)    # [(s,n), (t, q, r)]
            statsALL = big.tile([128, NT * 2], f32)       # (t, [mean, var])
            convrawALL = big.tile([128, NT * CONV_DIM], f32)
            convsALL = big.tile([128, NT * 64], f32)     # [r, (t, 64)]
            cvTALL = big.tile([64, NT * 128], f32)
            gsmTALL = big.tile([17, NT * 128], bf16)
            CsbALL = big.tile([128, NT * N_OUT], f32)
            gALL = big.tile([128, NT * NM], f32)
            lgALL = big.tile([128, NT * NM], f32)
            E1 = big.tile([128, NT * NM], f32)
            vt = big.tile([128, NT * NM], f32)
            E2 = big.tile([128, NT * NM], f32)
            EmALL = big.tile([128, NT * NM], f32)
            gsm17 = big.tile([128, NT * 17], f32)
            gsm17b = big.tile([128, NT * 17], bf16)
            st16a = big.tile([128, NT], f32)   # vstab
            stdALL = big.tile([128, NT], f32)
            rstdALL = big.tile([128, NT], f32)
            negmean = big.tile([128, NT], f32)
            s1g = big.tile([128, NT], f32)
            r1g = big.tile([128, NT], f32)
            mx1 = big.tile([128, NT], f32)
            mx2 = big.tile([128, NT], f32)
            E2m = big.tile([128, NT * NM], f32)
            E2b = big.tile([128, NT * NM], f32)
            msk1 = big.tile([128, NT * NM], f32)
            mskf = big.tile([128, NT * NM], f32)
            tmp16 = big.tile([128, NT], f32)

            meanv = statsALL[:].rearrange("p (t u) -> p t u", u=2)[:, :, 0]
            varv = statsALL[:].rearrange("p (t u) -> p t u", u=2)[:, :, 1]

            HT = NT // 2  # tiles per half

            def emit_A(t, pA):
                xts = xtALL[:, t * SEQ : (t + 1) * SEQ]
                xraw = xin.tile([128, 2 * 4 * C], f32, tag="xraw")
                xrv = xraw[:].rearrange("p (h j c) -> p h j c", h=2, j=4)
                nc.sync.dma_start(
                    xrv,
                    x_d[2 * t : 2 * t + 2].rearrange(
                        "h (j p) c -> p h j c", p=128
                    ),
                )
                for h in range(2):
                    psx = pA.tile([64, SEQ], f32, tag="psx")
                    for j in range(4):
                        nc.tensor.transpose(
                            psx[:, j * 128 : (j + 1) * 128], xrv[:, h, j],
                            ident_f[:],
                        )
                    if h == 0:
                        nc.vector.tensor_copy(xts[0:64, :], psx[:])
                    else:
                        nc.scalar.copy(xts[64:128, :], psx[:])

                bs6 = small.tile([128, 6], f32, tag="bs6")
                nc.vector.bn_stats(bs6[:], xts)
                nc.vector.bn_aggr(statsALL[:, 2 * t : 2 * t + 2], bs6[:])

                cvt = cvp.tile([128, CONV_DIM * KSZ], f32, tag="cvt")
                nc.gpsimd.tensor_tensor(
                    cvt[:].rearrange("p (d k) -> p d k", k=KSZ),
                    _win_ap(bass, xtALL[:], t * SEQ, NT * SEQ),
                    cw_t[:].unsqueeze(1).broadcast_to([128, CONV_DIM, KSZ]),
                    AL.mult,
                )
                nc.vector.tensor_reduce(
                    convrawALL[:, t * CONV_DIM : (t + 1) * CONV_DIM],
                    cvt[:].rearrange("p (d k) -> p d k", k=KSZ),
                    axis=AX.X, op=AL.add,
                )

                nc.scalar.copy(
                    xnpALL[:, t * SEQ : (t + 1) * SEQ].rearrange(
                        "p (q s n) -> p q s n", q=4, s=4
                    ),
                    xts.rearrange("p (n q s) -> p n q s", n=N_IN, q=4)
                    .rearrange("p n q s -> p q s n"),
                )
                nc.scalar.dma_start_transpose(
                    xnpT[:, t * SEQ : (t + 1) * SEQ].rearrange(
                        "p (q r) -> p q r", q=4
                    ),
                    xnpALL[:, t * SEQ : (t + 1) * SEQ],
                )

            def emit_gates(half):
                ts, te = half * HT, (half + 1) * HT
                tsl = slice(ts, te)
                sl8 = slice(ts * NM, te * NM)
                meanh = statsALL[:].rearrange("p (t u) -> p t u", u=2)[:, tsl, 0]
                varh = statsALL[:].rearrange("p (t u) -> p t u", u=2)[:, tsl, 1]
                sth = st16a[:, tsl]
                nc.vector.tensor_scalar(sth, varh, 1e-10, None, AL.add)
                nc.scalar.activation(stdALL[:, tsl], sth, AF.Sqrt)
                nc.vector.reciprocal(rstdALL[:, tsl], stdALL[:, tsl])
                nc.vector.tensor_scalar(
                    negmean[:, tsl], meanh, -1.0, None, AL.mult
                )

                convs_v = convsALL[:].rearrange("p (t u) -> p t u", u=64)[:, tsl]
                nc.vector.tensor_tensor(
                    convs_v[:, :, 0:CONV_DIM],
                    convrawALL[:].rearrange("p (t d) -> p t d", d=CONV_DIM)[
                        :, tsl
                    ],
                    rstdALL[:, tsl].unsqueeze(-1).broadcast_to(
                        [128, HT, CONV_DIM]
                    ),
                    AL.mult,
                )
                nc.vector.tensor_tensor(
                    tmp16[:, tsl], meanh, rstdALL[:, tsl], AL.mult
                )
                nc.vector.scalar_tensor_tensor(
                    convs_v[:, :, CONV_DIM : CONV_DIM + 1].squeeze(-1),
                    tmp16[:, tsl], negSw[:],
                    cb_t[:].broadcast_to([128, HT]),
                    AL.mult, AL.add,
                )
                with tc.tile_pool(name="pgate", bufs=1, space="PSUM") as pgate:
                    gpsum = pgate.tile([128, HT * NM], f32, name=f"gp{half}")
                    with tc.tile_pool(name="pB", bufs=1, space="PSUM") as pB:
                        psB = pB.tile([64, HT * 128], f32, name=f"psB{half}")
                        for i, t in enumerate(range(ts, te)):
                            nc.tensor.transpose(
                                psB[:, i * 128 : (i + 1) * 128],
                                convsALL[:, t * 64 : (t + 1) * 64],
                                ident_f[:],
                            )
                        nc.vector.tensor_copy(
                            cvTALL[:, ts * 128 : te * 128], psB[:]
                        )
                        for i, t in enumerate(range(ts, te)):
                            nc.tensor.matmul(
                                gpsum[:, i * NM : (i + 1) * NM],
                                cvTALL[:, t * 128 : (t + 1) * 128],
                                gstack[:],
                                start=True, stop=True,
                            )
                    lg_v = lgALL[:, sl8].rearrange("p (t m) -> p t m", m=NM)
                    nc.vector.tensor_tensor(
                        lg_v, gpsum[:].rearrange("p (t m) -> p t m", m=NM),
                        gb8[:].unsqueeze(1).broadcast_to([128, HT, NM]),
                        AL.add,
                    )
                nc.scalar.activation(E1[:, sl8], lgALL[:, sl8], AF.Exp)
                E1_v = E1[:, sl8].rearrange("p (t m) -> p t m", m=NM)
                nc.vector.tensor_reduce(s1g[:, tsl], E1_v, axis=AX.X, op=AL.add)
                nc.vector.reciprocal(r1g[:, tsl], s1g[:, tsl])
                vt_v = vt[:, sl8].rearrange("p (t m) -> p t m", m=NM)
                nc.vector.tensor_tensor(
                    vt_v, E1_v,
                    r1g[:, tsl].unsqueeze(-1).broadcast_to([128, HT, NM]),
                    AL.mult,
                )
                nc.scalar.activation(E2[:, sl8], vt[:, sl8], AF.Exp)
                E2_v = E2[:, sl8].rearrange("p (t m) -> p t m", m=NM)
                nc.vector.tensor_reduce(mx1[:, tsl], E2_v, axis=AX.X, op=AL.max)
                msk1_v = msk1[:, sl8].rearrange("p (t m) -> p t m", m=NM)
                nc.vector.tensor_tensor(
                    msk1_v, E2_v,
                    mx1[:, tsl].unsqueeze(-1).broadcast_to([128, HT, NM]),
                    AL.is_ge,
                )
                E2m_v = E2m[:, sl8].rearrange("p (t m) -> p t m", m=NM)
                nc.vector.tensor_tensor(E2m_v, E2_v, msk1_v, AL.mult)
                E2b_v = E2b[:, sl8].rearrange("p (t m) -> p t m", m=NM)
                nc.vector.tensor_tensor(E2b_v, E2_v, E2m_v, AL.subtract)
                nc.vector.tensor_reduce(mx2[:, tsl], E2b_v, axis=AX.X, op=AL.max)
                mskf_v = mskf[:, sl8].rearrange("p (t m) -> p t m", m=NM)
                nc.vector.tensor_tensor(
                    mskf_v, E2_v,
                    mx2[:, tsl].unsqueeze(-1).broadcast_to([128, HT, NM]),
                    AL.is_ge,
                )
                Em_v = EmALL[:, sl8].rearrange("p (t m) -> p t m", m=NM)
                nc.vector.tensor_tensor(Em_v, E2_v, mskf_v, AL.mult)
                nc.vector.tensor_reduce(s1g[:, tsl], Em_v, axis=AX.X, op=AL.add)
                nc.vector.reciprocal(r1g[:, tsl], s1g[:, tsl])
                g_v = gALL[:, sl8].rearrange("p (t m) -> p t m", m=NM)
                nc.vector.tensor_tensor(
                    g_v, Em_v,
                    r1g[:, tsl].unsqueeze(-1).broadcast_to([128, HT, NM]),
                    AL.mult,
                )
                g17 = gsm17[:].rearrange("p (t u) -> p t u", u=17)[:, tsl]
                nc.vector.tensor_tensor(
                    g17[:, :, 0:NM], g_v,
                    negmean[:, tsl].unsqueeze(-1).broadcast_to([128, HT, NM]),
                    AL.mult,
                )
                nc.vector.tensor_tensor(
                    g17[:, :, NM : 2 * NM], g_v,
                    stdALL[:, tsl].unsqueeze(-1).broadcast_to([128, HT, NM]),
                    AL.mult,
                )
                nc.vector.tensor_copy(
                    g17[:, :, 16:17],
                    statsALL[:].rearrange("p (t u) -> p t u", u=2)[:, tsl, 0:1],
                )
                nc.vector.tensor_copy(
                    gsm17b[:, ts * 17 : te * 17], gsm17[:, ts * 17 : te * 17]
                )
                with tc.tile_pool(name="pI2", bufs=1, space="PSUM") as pI2:
                    psI = pI2.tile([17, HT * 128], bf16, name=f"psI{half}")
                    for i, t in enumerate(range(ts, te)):
                        nc.tensor.transpose(
                            psI[:, i * 128 : (i + 1) * 128],
                            gsm17b[:, t * 17 : (t + 1) * 17],
                            ident_m[:],
                        )
                    nc.vector.tensor_copy(
                        gsmTALL[:, ts * 128 : te * 128], psI[:]
                    )
                    Cps = pI2.tile([128, HT * N_OUT], f32, name=f"Cps{half}")
                    for i, t in enumerate(range(ts, te)):
                        nc.tensor.matmul(
                            Cps[:, i * N_OUT : (i + 1) * N_OUT],
                            gsmTALL[:, t * 128 : (t + 1) * 128],
                            Crhsb[:],
                            start=True, stop=True,
                        )
                    nc.scalar.copy(
                        CsbALL[:, ts * N_OUT : te * N_OUT], Cps[:]
                    )

            def emit_experts(t, qp):
                # 8 psum units of 2 segments each: unit u2 covers u = 2u2, 2u2+1
                acc = accp.tile([128, PRED], f32, tag="acc", bufs=4)
                ysb = accp.tile([128, 4 * 4 * 360], bf16, tag="ysb", bufs=5)
                for u2 in range(8):
                    q = u2 // 2
                    yq = qp.tile([128, 2, 512], f32, tag="q")
                    lhsT = xnpT[:, (t * 4 + q) * 128 : (t * 4 + q + 1) * 128]
                    for sp in range(2):
                        s_loc = 2 * (u2 % 2) + sp
                        nc.tensor.matmul(
                            yq[:, sp, 0 : NM * N_OUT],
                            lhsT,
                            W4[:, s_loc * 360 : (s_loc + 1) * 360],
                            start=True, stop=True,
                        )
                    nc.scalar.copy(
                        ysb[:, u2 * 720 : (u2 + 1) * 720].rearrange(
                            "p (s mo) -> p s mo", s=2
                        ),
                        yq[:, :, 0:360],
                    )
                return acc, ysb

            def emit_madd(t, acc, ysb, m):
                # acc[p, 45u+o] = sum_m g_m ysb[p, 360u+45m+o] + C[p,o]
                dst = bass.AP(
                    acc.tensor, acc.offset,
                    [[PRED, 128], [N_OUT, SEG], [1, N_OUT]],
                )
                src = bass.AP(
                    ysb.tensor, ysb.offset + m * N_OUT,
                    [[4 * 1440, 128], [360, SEG], [1, N_OUT]],
                )
                gm = gALL[:, t * NM + m : t * NM + m + 1]
                if m == 0:
                    Cb = (
                        CsbALL[:, t * N_OUT : (t + 1) * N_OUT]
                        .unsqueeze(1)
                        .broadcast_to([128, SEG, N_OUT])
                    )
                    nc.vector.scalar_tensor_tensor(
                        dst, src, gm, Cb, AL.mult, AL.add
                    )
                else:
                    nc.vector.scalar_tensor_tensor(
                        dst, src, gm, dst, AL.mult, AL.add
                    )

            def emit_out(t, acc, qp):
                # 8 transposes of 90-col blocks: block b = segments (2b, 2b+1)
                pox = qp.tile([128, 2, 512], f32, tag="q")
                poxv = bass.AP(
                    pox.tensor, pox.offset, [[1024, 90], [128, 8], [1, 128]]
                )
                accv = acc[:].rearrange("p (b w) -> p b w", b=8)
                for b in range(8):
                    nc.tensor.transpose(
                        poxv[:, b : b + 1].squeeze(1), accv[:, b], ident_f[:]
                    )
                ocs = ocsp.tile([90, 8 * 128], f32, tag="ocs")
                nc.scalar.copy(ocs[:, 0:512], poxv[:, 0:4])
                nc.scalar.copy(ocs[:, 512:1024], poxv[:, 4:8])
                ocsv = ocs[:].rearrange("p (b h c) -> p b h c", b=8, h=2)
                for h in range(2):
                    dstv = out_d[2 * t + h].rearrange(
                        "(o b u) c -> o b u c", o=N_OUT, b=8
                    )
                    for u2 in range(2):
                        nc.sync.dma_start(
                            dstv[:, :, u2, :],
                            ocsv[45 * u2 : 45 * (u2 + 1), :, h, :],
                        )

            def emit_C_phase(tiles, qp, interleave_A=None):
                # software-pipelined: out(pair p) is emitted AFTER
                # experts(pair p+1) so Act's in-order queue never blocks
                # the next pair's evacuations behind ocs copies.
                pairs = [(tiles[2 * i], tiles[2 * i + 1])
                         for i in range(len(tiles) // 2)]
                state = {}
                for i, (t0, t1) in enumerate(pairs):
                    state[i] = (
                        emit_experts(t0, qp), emit_experts(t1, qp), t0, t1
                    )
                    if i > 0:
                        (pa0, pa1, pt0, pt1) = state.pop(i - 1)
                        emit_out(pt0, pa0[0], qp)
                        emit_out(pt1, pa1[0], qp)
                    (a0, a1, _, _) = state[i]
                    for m in range(NM):
                        emit_madd(t0, a0[0], a0[1], m)
                        emit_madd(t1, a1[0], a1[1], m)
                    if interleave_A is not None:
                        emit_A(interleave_A[2 * i], pA)
                        emit_A(interleave_A[2 * i + 1], pA)
                (pa0, pa1, pt0, pt1) = state.pop(len(pairs) - 1)
                emit_out(pt0, pa0[0], qp)
                emit_out(pt1, pa1[0], qp)

            # ================= schedule =================
            with tc.tile_pool(name="pA", bufs=2, space="PSUM") as pA:
                for t in range(HT):
                    emit_A(t, pA)
                emit_gates(0)
                with tc.tile_pool(name="qp0", bufs=3, space="PSUM") as qp0:
                    emit_C_phase(
                        list(range(HT)), qp0,
                        interleave_A=list(range(HT, NT)),
                    )
                emit_gates(1)
                with tc.tile_pool(name="qp1", bufs=3, space="PSUM") as qp1:
                    emit_C_phase(list(range(HT, NT)), qp1)

    nc.compile()
    return nc


def _get_program():
    if "v2" not in _CACHE:
        _CACHE["v2"] = _build_program()
    return _CACHE["v2"]


def kernel(x, conv_w, conv_b, gate_w, gate_b, map_w, map_b, _mm_dt="bfloat16",
           _trace=False):
    from concourse.bass_utils import run_bass_kernel_spmd

    nc = _get_program()
    x = np.ascontiguousarray(np.asarray(x, dtype=np.float32))
    params = dict(
        conv_w=np.ascontiguousarray(np.asarray(conv_w, np.float32)),
        conv_b=np.ascontiguousarray(np.asarray(conv_b, np.float32)),
        gate_w=np.ascontiguousarray(np.asarray(gate_w, np.float32)),
        gate_b=np.ascontiguousarray(np.asarray(gate_b, np.float32)),
        map_w=np.ascontiguousarray(np.asarray(map_w, np.float32)),
        map_b=np.ascontiguousarray(np.asarray(map_b, np.float32)),
    )
    in_maps = [
        dict(x=x[i * BPC : (i + 1) * BPC], **params) for i in range(NCORES)
    ]
    res = run_bass_kernel_spmd(
        nc, in_maps, core_ids=list(range(NCORES)), trace=_trace
    )
    out = np.concatenate([res.results[i]["out"] for i in range(NCORES)], axis=0)
    if _trace:
        return out, res
    return out


# revision 23
# speedup vs baseline: 1.2058x; 1.0307x over previous
"""Trainium2 Bass kernel for nn_CMoSModel (moe_routing) - v2.

Data-parallel over batch: bs=256 -> 32 per core on 8 cores; params replicated.

Math (per row r=(b,c), L=512):
  mean/var over L; xn=(xt-mean)*rstd
  conv = depthwise(xn,k=16,s=8)+cb -> gates = top2-renorm softmax chain  [r,8]
  y_m = xn.seg @ map_w[m] + map_b[m];  out = (sum_m g_m y_m)*std + mean

Key identity used here: std*rstd = 1, so
  out[r,(o,s)] = sum_m g_m * yraw_m[r,(o,s)] + C[r,o]
  yraw_m = raw xt through expert weights (no normalize!)
  C[r,o] = -mean*sum_m g_m W1[m,o] + std*sum_m g_m b_m[o] + mean,
  W1[m,o] = sum_n map_w[m,o,n]

Structure (per core, 16 tiles of 128 rows = 2 batches x 64 ch):
  Pass A: load + PE-transpose input -> xt (bf16); bn_stats/aggr stats;
          depthwise conv as ONE windowed tensor_tensor (GPSIMD) + grouped
          tensor_reduce (DVE); Act writes permuted bf16 xnp; XBAR
          dma_start_transpose makes xnpT (matmul lhsT) - no PSUM round trip.
  I1/I2 (batched over all 16 tiles in single wide ops): std/rstd; gate
          logits += gb; double softmax + top-2 via grouped reduces and
          stride-0 broadcast APs; gsm17; per-tile correction C via tiny
          PE matmuls (17x45 rhs).
  Pass C: per tile 16 big matmuls (block-diag W4 [128,4*360] bf16, 360 cols
          each, full-K contraction) -> dense y in PSUM; combine = 8
          scalar_tensor_tensor madds per q split DVE/GPSIMD; output PE
          transposes reuse the same 8-bank PSUM pool; stores via gpsimd DMA.
"""

import sys

import numpy as np

for p in ("/opt/trn_rl_repo", "/opt/pypackages"):
    if p not in sys.path:
        sys.path.insert(0, p)

BS = 256
SEQ = 512
PRED = 720
C = 64
SEG = 16
NM = 8
KSZ = 16
STRIDE = 8
CONV_DIM = 63
N_IN = 32
N_OUT = 45
NCORES = 8
BPC = BS // NCORES   # 32 batches per core
NT = BPC // 2        # 16 tiles, 2 batches each

_CACHE = {}


def _win_ap(bass, tile_ap, col_off, part_stride):
    """Overlapping conv window AP: [p, d=63 (stride 8), k=16 (stride 1)]."""
    return bass.AP(
        tile_ap.tensor,
        tile_ap.offset + col_off,
        [[part_stride, 128], [STRIDE, CONV_DIM], [1, KSZ]],
    )


def _build_program():
    import concourse.bass as bass
    import concourse.tile as tile
    from concourse import bacc
    from concourse import mybir
    from concourse.masks import make_identity

    f32 = mybir.dt.float32
    bf16 = mybir.dt.bfloat16
    AL = mybir.AluOpType
    AF = mybir.ActivationFunctionType
    AX = mybir.AxisListType

    nc = bacc.Bacc(None, target_bir_lowering=False)
    x_d = nc.declare_dram_parameter("x", [BPC, SEQ, C], f32, isOutput=False)
    cw_d = nc.declare_dram_parameter("conv_w", [C, 1, KSZ], f32, isOutput=False)
    cb_d = nc.declare_dram_parameter("conv_b", [C], f32, isOutput=False)
    gw_d = nc.declare_dram_parameter("gate_w", [NM, CONV_DIM], f32, isOutput=False)
    gb_d = nc.declare_dram_parameter("gate_b", [NM], f32, isOutput=False)
    mw_d = nc.declare_dram_parameter("map_w", [NM, N_OUT, N_IN], f32, isOutput=False)
    mb_d = nc.declare_dram_parameter("map_b", [NM, N_OUT], f32, isOutput=False)
    out_d = nc.declare_dram_parameter("out", [BPC, PRED, C], f32, isOutput=True)

    with tile.TileContext(nc) as tc:
        with (
            tc.tile_pool(name="consts", bufs=1) as consts,
            tc.tile_pool(name="big", bufs=1) as big,
            tc.tile_pool(name="xin", bufs=5) as xin,
            tc.tile_pool(name="cvp", bufs=2) as cvp,
            tc.tile_pool(name="small", bufs=3) as small,
            tc.tile_pool(name="accp", bufs=2) as accp,
            tc.tile_pool(name="ocsp", bufs=3) as ocsp,
        ):
            # ---------------- constants ----------------
            zero_t = consts.tile([128, 1], f32)
            nc.gpsimd.memset(zero_t[:], 0.0)
            nc.const_aps.aps[(f32, 0.0)] = zero_t[:]

            ident_f = consts.tile([128, 128], f32)
            make_identity(nc, ident_f[:])
            ident_m = consts.tile([128, 128], bf16)
            make_identity(nc, ident_m[:])

            # conv weights per-channel, dup 2x over h
            cw_t = consts.tile([128, KSZ], f32)
            nc.sync.dma_start(cw_t[0:64, :], cw_d[:, 0, :])
            nc.sync.dma_start(cw_t[64:128, :], cw_d[:, 0, :])
            cb_t = consts.tile([128, 1], f32)
            nc.sync.dma_start(cb_t[0:64, :], cb_d[:, None])
            nc.sync.dma_start(cb_t[64:128, :], cb_d[:, None])
            negSw = consts.tile([128, 1], f32)
            nc.vector.tensor_reduce(negSw[:], cw_t[:], axis=AX.X, op=AL.add)
            nc.vector.tensor_scalar(negSw[:], negSw[:], -1.0, None, AL.mult)

            # gate weights: gstack [64, 8] = [gw.T (63 rows); Gsum (1 row)]
            gw_f = consts.tile([CONV_DIM, NM], f32)
            nc.sync.dma_start(gw_f[:, :], gw_d[:].rearrange("m d -> d m"))
            gstack = consts.tile([64, NM], f32)
            nc.vector.tensor_copy(gstack[0:CONV_DIM, :], gw_f[:])
            gw_row = consts.tile([1, NM * CONV_DIM], f32)
            nc.sync.dma_start(
                gw_row[:, :], gw_d[:].rearrange("m d -> (m d)")[None, :]
            )
            gsum_r = consts.tile([1, NM], f32)
            nc.vector.tensor_reduce(
                gsum_r[:],
                gw_row[:].rearrange("p (m d) -> p m d", m=NM),
                axis=AX.X, op=AL.add,
            )
            nc.sync.dma_start(gstack[CONV_DIM:64, :], gsum_r[:])

            gb8 = consts.tile([128, NM], f32)
            nc.sync.dma_start(gb8[:, :], gb_d[None, :].broadcast_to([128, NM]))

            # W4 block-diag [128=(s,n), 4*360=(s',m,o)] bf16
            W4f = consts.tile([128, 4 * NM * N_OUT], f32)
            nc.vector.memset(W4f[:], 0.0)
            for s in range(4):
                nc.sync.dma_start(
                    W4f[32 * s : 32 * (s + 1),
                        360 * s : 360 * (s + 1)],
                    mw_d[:].rearrange("m o n -> n (m o)"),
                )
            W4 = consts.tile([128, 4 * NM * N_OUT], bf16)
            nc.vector.tensor_copy(W4[:], W4f[:])

            # Crhs [17, 45] = [W1 (8); map_b (8); ones (1)]
            mwn = consts.tile([NM, N_OUT * N_IN], f32)
            nc.sync.dma_start(mwn[:, :], mw_d[:].rearrange("m o n -> m (o n)"))
            Crhs_f = consts.tile([17, N_OUT], f32)
            nc.vector.memset(Crhs_f[:], 1.0)
            nc.vector.tensor_reduce(
                Crhs_f[0:NM, :],
                mwn[:].rearrange("p (o n) -> p o n", o=N_OUT),
                axis=AX.X, op=AL.add,
            )
            nc.sync.dma_start(Crhs_f[NM : 2 * NM, :], mb_d[:, :])
            Crhsb = consts.tile([17, N_OUT], bf16)
            nc.vector.tensor_copy(Crhsb[:], Crhs_f[:])

            # ---------------- big SBUF tensors ----------------
            xtALL = big.tile([128, NT * SEQ], f32)       # [r, (t, l)]
            xnpALL = big.tile([128, NT * SEQ], bf16)      # [r, (t, q, s, n)]
            xnpT = big.tile([128, NT * 4 * 128], bf16)    # [(s,n), (t, q, r)]
            statsALL = big.tile([128, NT * 2], f32)       # (t, [mean, var])
            convrawALL = big.tile([128, NT * CONV_DIM], f32)
            convsALL = big.tile([128, NT * 64], f32)     # [r, (t, 64)]
            cvTALL = big.tile([64, NT * 128], f32)
            gsmTALL = big.tile([17, NT * 128], bf16)
            CsbALL = big.tile([128, NT * N_OUT], f32)
            gALL = big.tile([128, NT * NM], f32)
            lgALL = big.tile([128, NT * NM], f32)
            E1 = big.tile([128, NT * NM], f32)
            vt = big.tile([128, NT * NM], f32)
            E2 = big.tile([128, NT * NM], f32)
            EmALL = big.tile([128, NT * NM], f32)
            gsm17 = big.tile([128, NT * 17], f32)
            gsm17b = big.tile([128, NT * 17], bf16)
            st16a = big.tile([128, NT], f32)   # vstab
            stdALL = big.tile([128, NT], f32)
            rstdALL = big.tile([128, NT], f32)
            negmean = big.tile([128, NT], f32)
            s1g = big.tile([128, NT], f32)
            r1g = big.tile([128, NT], f32)
            mx1 = big.tile([128, NT], f32)
            mx2 = big.tile([128, NT], f32)
            E2m = big.tile([128, NT * NM], f32)
            E2b = big.tile([128, NT * NM], f32)
            msk1 = big.tile([128, NT * NM], f32)
            mskf = big.tile([128, NT * NM], f32)
            tmp16 = big.tile([128, NT], f32)

            meanv = statsALL[:].rearrange("p (t u) -> p t u", u=2)[:, :, 0]
            varv = statsALL[:].rearrange("p (t u) -> p t u", u=2)[:, :, 1]

            HT = NT // 2  # tiles per half

            def emit_A(t, pA):
                xts = xtALL[:, t * SEQ : (t + 1) * SEQ]
                xraw = xin.tile([128, 2 * 4 * C], f32, tag="xraw")
                xrv = xraw[:].rearrange("p (h j c) -> p h j c", h=2, j=4)
                nc.sync.dma_start(
                    xrv,
                    x_d[2 * t : 2 * t + 2].rearrange(
                        "h (j p) c -> p h j c", p=128
                    ),
                )
                for h in range(2):
                    psx = pA.tile([64, SEQ], f32, tag="psx")
                    for j in range(4):
                        nc.tensor.transpose(
                            psx[:, j * 128 : (j + 1) * 128], xrv[:, h, j],
                            ident_f[:],
                        )
                    if h == 0:
                        nc.vector.tensor_copy(xts[0:64, :], psx[:])
                    else:
                        nc.scalar.copy(xts[64:128, :], psx[:])

                bs6 = small.tile([128, 6], f32, tag="bs6")
                nc.vector.bn_stats(bs6[:], xts)
                nc.vector.bn_aggr(statsALL[:, 2 * t : 2 * t + 2], bs6[:])

                cvt = cvp.tile([128, CONV_DIM * KSZ], f32, tag="cvt")
                nc.gpsimd.tensor_tensor(
                    cvt[:].rearrange("p (d k) -> p d k", k=KSZ),
                    _win_ap(bass, xtALL[:], t * SEQ, NT * SEQ),
                    cw_t[:].unsqueeze(1).broadcast_to([128, CONV_DIM, KSZ]),
                    AL.mult,
                )
                nc.vector.tensor_reduce(
                    convrawALL[:, t * CONV_DIM : (t + 1) * CONV_DIM],
                    cvt[:].rearrange("p (d k) -> p d k", k=KSZ),
                    axis=AX.X, op=AL.add,
                )

                nc.scalar.copy(
                    xnpALL[:, t * SEQ : (t + 1) * SEQ].rearrange(
                        "p (q s n) -> p q s n", q=4, s=4
                    ),
                    xts.rearrange("p (n q s) -> p n q s", n=N_IN, q=4)
                    .rearrange("p n q s -> p q s n"),
                )
                nc.scalar.dma_start_transpose(
                    xnpT[:, t * SEQ : (t + 1) * SEQ].rearrange(
                        "p (q r) -> p q r", q=4
                    ),
                    xnpALL[:, t * SEQ : (t + 1) * SEQ],
                )

            def emit_gates(half):
                ts, te = half * HT, (half + 1) * HT
                tsl = slice(ts, te)
                sl8 = slice(ts * NM, te * NM)
                meanh = statsALL[:].rearrange("p (t u) -> p t u", u=2)[:, tsl, 0]
                varh = statsALL[:].rearrange("p (t u) -> p t u", u=2)[:, tsl, 1]
                sth = st16a[:, tsl]
                nc.vector.tensor_scalar(sth, varh, 1e-10, None, AL.add)
                nc.scalar.activation(stdALL[:, tsl], sth, AF.Sqrt)
                nc.vector.reciprocal(rstdALL[:, tsl], stdALL[:, tsl])
                nc.vector.tensor_scalar(
                    negmean[:, tsl], meanh, -1.0, None, AL.mult
                )

                convs_v = convsALL[:].rearrange("p (t u) -> p t u", u=64)[:, tsl]
                nc.vector.tensor_tensor(
                    convs_v[:, :, 0:CONV_DIM],
                    convrawALL[:].rearrange("p (t d) -> p t d", d=CONV_DIM)[
                        :, tsl
                    ],
                    rstdALL[:, tsl].unsqueeze(-1).broadcast_to(
                        [128, HT, CONV_DIM]
                    ),
                    AL.mult,
                )
                nc.vector.tensor_tensor(
                    tmp16[:, tsl], meanh, rstdALL[:, tsl], AL.mult
                )
                nc.vector.scalar_tensor_tensor(
                    convs_v[:, :, CONV_DIM : CONV_DIM + 1].squeeze(-1),
                    tmp16[:, tsl], negSw[:],
                    cb_t[:].broadcast_to([128, HT]),
                    AL.mult, AL.add,
                )
                with tc.tile_pool(name="pgate", bufs=1, space="PSUM") as pgate:
                    gpsum = pgate.tile([128, HT * NM], f32, name=f"gp{half}")
                    with tc.tile_pool(name="pB", bufs=1, space="PSUM") as pB:
                        psB = pB.tile([64, HT * 128], f32, name=f"psB{half}")
                        for i, t in enumerate(range(ts, te)):
                            nc.tensor.transpose(
                                psB[:, i * 128 : (i + 1) * 128],
                                convsALL[:, t * 64 : (t + 1) * 64],
                                ident_f[:],
                            )
                        nc.vector.tensor_copy(
                            cvTALL[:, ts * 128 : te * 128], psB[:]
                        )
                        for i, t in enumerate(range(ts, te)):
                            nc.tensor.matmul(
                                gpsum[:, i * NM : (i + 1) * NM],
                                cvTALL[:, t * 128 : (t + 1) * 128],
                                gstack[:],
                                start=True, stop=True,
                            )
                    lg_v = lgALL[:, sl8].rearrange("p (t m) -> p t m", m=NM)
                    nc.vector.tensor_tensor(
                        lg_v, gpsum[:].rearrange("p (t m) -> p t m", m=NM),
                        gb8[:].unsqueeze(1).broadcast_to([128, HT, NM]),
                        AL.add,
                    )
                nc.scalar.activation(E1[:, sl8], lgALL[:, sl8], AF.Exp)
                E1_v = E1[:, sl8].rearrange("p (t m) -> p t m", m=NM)
                nc.vector.tensor_reduce(s1g[:, tsl], E1_v, axis=AX.X, op=AL.add)
                nc.vector.reciprocal(r1g[:, tsl], s1g[:, tsl])
                vt_v = vt[:, sl8].rearrange("p (t m) -> p t m", m=NM)
                nc.vector.tensor_tensor(
                    vt_v, E1_v,
                    r1g[:, tsl].unsqueeze(-1).broadcast_to([128, HT, NM]),
                    AL.mult,
                )
                nc.scalar.activation(E2[:, sl8], vt[:, sl8], AF.Exp)
                E2_v = E2[:, sl8].rearrange("p (t m) -> p t m", m=NM)
                nc.vector.tensor_reduce(mx1[:, tsl], E2_v, axis=AX.X, op=AL.max)
                msk1_v = msk1[:, sl8].rearrange("p (t m) -> p t m", m=NM)
                nc.vector.tensor_tensor(
                    msk1_v, E2_v,
                    mx1[:, tsl].unsqueeze(-1).broadcast_to([128, HT, NM]),
                    AL.is_ge,
                )
                E2m_v = E2m[:, sl8].rearrange("p (t m) -> p t m", m=NM)
                nc.vector.tensor_tensor(E2m_v, E2_v, msk1_v, AL.mult)
                E2b_v = E2b[:, sl8].rearrange("p (t m) -> p t m", m=NM)
                nc.vector.tensor_tensor(E2b_v, E2_v, E2m_v, AL.subtract)
                nc.vector.tensor_reduce(mx2[:, tsl], E2b_v, axis=AX.X, op=AL.max)
                mskf_v = mskf[:, sl8].rearrange("p (t m) -> p t m", m=NM)
                nc.vector.tensor_tensor(
                    mskf_v, E2_v,
                    mx2[:, tsl].unsqueeze(-1).broadcast_to([128, HT, NM]),
                    AL.is_ge,
                )
                Em_v = EmALL[:, sl8].rearrange("p (t m) -> p t m", m=NM)
                nc.vector.tensor_tensor(Em_v, E2_v, mskf_v, AL.mult)
                nc.vector.tensor_reduce(s1g[:, tsl], Em_v, axis=AX.X, op=AL.add)
                nc.vector.reciprocal(r1g[:, tsl], s1g[:, tsl])
                g_v = gALL[:, sl8].rearrange("p (t m) -> p t m", m=NM)
                nc.vector.tensor_tensor(
                    g_v, Em_v,
                    r1g[:, tsl].unsqueeze(-1).broadcast_to([128, HT, NM]),
                    AL.mult,
                )
                g17 = gsm17[:].rearrange("p (t u) -> p t u", u=17)[:, tsl]
                nc.vector.tensor_tensor(
                    g17[:, :, 0:NM], g_v,
                    negmean[:, tsl].unsqueeze(-1).broadcast_to([128, HT, NM]),
                    AL.mult,
                )
                nc.vector.tensor_tensor(
                    g17[:, :, NM : 2 * NM], g_v,
                    stdALL[:, tsl].unsqueeze(-1).broadcast_to([128, HT, NM]),
                    AL.mult,
                )
                nc.vector.tensor_copy(
                    g17[:, :, 16:17],
                    statsALL[:].rearrange("p (t u) -> p t u", u=2)[:, tsl, 0:1],
                )
                nc.vector.tensor_copy(
                    gsm17b[:, ts * 17 : te * 17], gsm17[:, ts * 17 : te * 17]
                )
                with tc.tile_pool(name="pI2", bufs=1, space="PSUM") as pI2:
                    psI = pI2.tile([17, HT * 128], bf16, name=f"psI{half}")
                    for i, t in enumerate(range(ts, te)):
                        nc.tensor.transpose(
                            psI[:, i * 128 : (i + 1) * 128],
                            gsm17b[:, t * 17 : (t + 1) * 17],
                            ident_m[:],
                        )
                    nc.vector.tensor_copy(
                        gsmTALL[:, ts * 128 : te * 128], psI[:]
                    )
                    Cps = pI2.tile([128, HT * N_OUT], f32, name=f"Cps{half}")
                    for i, t in enumerate(range(ts, te)):
                        nc.tensor.matmul(
                            Cps[:, i * N_OUT : (i + 1) * N_OUT],
                            gsmTALL[:, t * 128 : (t + 1) * 128],
                            Crhsb[:],
                            start=True, stop=True,
                        )
                    nc.scalar.copy(
                        CsbALL[:, ts * N_OUT : te * N_OUT], Cps[:]
                    )

            def emit_experts(t, qp, wide=False):
                # psum units cover 2 (narrow) or 4 (wide) segments each
                acc = accp.tile([128, PRED], f32, tag="acc", bufs=4)
                ysb = accp.tile([128, 4 * 4 * 360], bf16, tag="ysb", bufs=4)
                nu, w = (4, 4) if wide else (8, 2)
                for u2 in range(nu):
                    q = u2 * w // 4
                    yq = qp.tile([128, w, 512], f32, tag="q")
                    lhsT = xnpT[:, (t * 4 + q) * 128 : (t * 4 + q + 1) * 128]
                    for sp in range(w):
                        s_loc = (u2 * w) % 4 + sp
                        nc.tensor.matmul(
                            yq[:, sp, 0 : NM * N_OUT],
                            lhsT,
                            W4[:, s_loc * 360 : (s_loc + 1) * 360],
                            start=True, stop=True,
                        )
                    nc.scalar.copy(
                        ysb[:, u2 * w * 360 : (u2 + 1) * w * 360].rearrange(
                            "p (s mo) -> p s mo", s=w
                        ),
                        yq[:, :, 0:360],
                    )
                return acc, ysb

            def emit_madd(t, acc, ysb, m):
                # acc[p, 45u+o] = sum_m g_m ysb[p, 360u+45m+o] + C[p,o]
                dst = bass.AP(
                    acc.tensor, acc.offset,
                    [[PRED, 128], [N_OUT, SEG], [1, N_OUT]],
                )
                src = bass.AP(
                    ysb.tensor, ysb.offset + m * N_OUT,
                    [[4 * 1440, 128], [360, SEG], [1, N_OUT]],
                )
                gm = gALL[:, t * NM + m : t * NM + m + 1]
                if m == 0:
                    Cb = (
                        CsbALL[:, t * N_OUT : (t + 1) * N_OUT]
                        .unsqueeze(1)
                        .broadcast_to([128, SEG, N_OUT])
                    )
                    nc.vector.scalar_tensor_tensor(
                        dst, src, gm, Cb, AL.mult, AL.add
                    )
                else:
                    nc.vector.scalar_tensor_tensor(
                        dst, src, gm, dst, AL.mult, AL.add
                    )

            def emit_out(t, acc, qp, wide=False):
                # 8 transposes of 90-col blocks: block b = segments (2b, 2b+1)
                w = 4 if wide else 2
                pox = qp.tile([128, w, 512], f32, tag="q")
                poxv = bass.AP(
                    pox.tensor, pox.offset, [[w * 512, 90], [128, 8], [1, 128]]
                )
                accv = acc[:].rearrange("p (b w) -> p b w", b=8)
                for b in range(8):
                    nc.tensor.transpose(
                        poxv[:, b : b + 1].squeeze(1), accv[:, b], ident_f[:]
                    )
                ocs = ocsp.tile([90, 8 * 128], f32, tag="ocs")
                nc.scalar.copy(ocs[:, 0:512], poxv[:, 0:4])
                nc.scalar.copy(ocs[:, 512:1024], poxv[:, 4:8])
                ocsv = ocs[:].rearrange("p (b h c) -> p b h c", b=8, h=2)
                for h in range(2):
                    dstv = out_d[2 * t + h].rearrange(
                        "(o b u) c -> o b u c", o=N_OUT, b=8
                    )
                    for u2 in range(2):
                        eng = nc.sync if u2 == 0 else nc.gpsimd
                        eng.dma_start(
                            dstv[:, :, u2, :],
                            ocsv[45 * u2 : 45 * (u2 + 1), :, h, :],
                        )

            def emit_C_phase(tiles, qp, interleave_A=None, wide=False):
                # software-pipelined: out(pair p) is emitted AFTER
                # experts(pair p+1) so Act's in-order queue never blocks
                # the next pair's evacuations behind ocs copies.
                pairs = [(tiles[2 * i], tiles[2 * i + 1])
                         for i in range(len(tiles) // 2)]
                state = {}
                for i, (t0, t1) in enumerate(pairs):
                    state[i] = (
                        emit_experts(t0, qp, wide), emit_experts(t1, qp, wide),
                        t0, t1,
                    )
                    if i > 0:
                        (pa0, pa1, pt0, pt1) = state.pop(i - 1)
                        emit_out(pt0, pa0[0], qp, wide)
                        emit_out(pt1, pa1[0], qp, wide)
                    (a0, a1, _, _) = state[i]
                    for m in range(NM):
                        emit_madd(t0, a0[0], a0[1], m)
                        emit_madd(t1, a1[0], a1[1], m)
                    if interleave_A is not None:
                        emit_A(interleave_A[2 * i], pA)
                        emit_A(interleave_A[2 * i + 1], pA)
                (pa0, pa1, pt0, pt1) = state.pop(len(pairs) - 1)
                emit_out(pt0, pa0[0], qp, wide)
                emit_out(pt1, pa1[0], qp, wide)

            # ================= schedule =================
            with tc.tile_pool(name="pA", bufs=2, space="PSUM") as pA:
                for t in range(HT):
                    emit_A(t, pA)
                emit_gates(0)
                with tc.tile_pool(name="qp0", bufs=3, space="PSUM") as qp0:
                    emit_C_phase(
                        list(range(HT)), qp0,
                        interleave_A=list(range(HT, NT)),
                    )
            emit_gates(1)
            with tc.tile_pool(name="qp1", bufs=2, space="PSUM") as qp1:
                emit_C_phase(list(range(HT, NT)), qp1, wide=True)

    nc.compile()
    return nc


def _get_program():
    if "v2" not in _CACHE:
        _CACHE["v2"] = _build_program()
    return _CACHE["v2"]


def kernel(x, conv_w, conv_b, gate_w, gate_b, map_w, map_b, _mm_dt="bfloat16",
           _trace=False):
    from concourse.bass_utils import run_bass_kernel_spmd

    nc = _get_program()
    x = np.ascontiguousarray(np.asarray(x, dtype=np.float32))
    params = dict(
        conv_w=np.ascontiguousarray(np.asarray(conv_w, np.float32)),
        conv_b=np.ascontiguousarray(np.asarray(conv_b, np.float32)),
        gate_w=np.ascontiguousarray(np.asarray(gate_w, np.float32)),
        gate_b=np.ascontiguousarray(np.asarray(gate_b, np.float32)),
        map_w=np.ascontiguousarray(np.asarray(map_w, np.float32)),
        map_b=np.ascontiguousarray(np.asarray(map_b, np.float32)),
    )
    in_maps = [
        dict(x=x[i * BPC : (i + 1) * BPC], **params) for i in range(NCORES)
    ]
    res = run_bass_kernel_spmd(
        nc, in_maps, core_ids=list(range(NCORES)), trace=_trace
    )
    out = np.concatenate([res.results[i]["out"] for i in range(NCORES)], axis=0)
    if _trace:
        return out, res
    return out
